# revision 15
# baseline (speedup 1.0000x reference)
"""Trainium2 Bass kernel for the DelayNetwork spiking RNN.

Strategy: tensor-parallel over the 2048 recurrent neurons across 8 cores
(256 neurons each).  Each core keeps its slice of the 4 delay-masked
recurrent weight matrices resident in SBUF, split into bf16 hi+lo pairs
(W = W_hi + W_lo); spikes are exactly representable in bf16, so two
full-rate bf16 matmuls reproduce the f32 matmul to ~1e-7 absolute — at
2x the fp32 PE rate (fp32 matmul runs at 4 cycles/row on TRN2, bf16 at
1).  The hi and lo halves are packed side-by-side in one N=512 rhs so
each k-tile costs a single LDWEIGHTS+MATMUL pair; the PSUM column halves
are summed in the membrane update.  W_i is split three ways (its values
are ~20x larger, so a 2-way split leaves ~2e-6 current error that flips
spikes).

Per step: spike-stationary matmuls (lhsT = gathered spike k-tile
(128,64) bf16) accumulate bias + input current + 4 delay taps into one
f32 PSUM group; a fused DVE chain updates the membrane and thresholds
spikes; the spike vector is PE-transposed to neuron-major, cast to bf16,
and AllGathered (d>=2 taps of the next step overlap the collective);
the spike raster streams to DRAM.  The output projection is deferred:
mem_out(T) = sum_t c^(T-1-t) (z_t @ W_o^T + B_o), accumulated as
wz = wz*c + z_t per step with a single 2-matmul fp32 projection at the
end; the 8 partial (128,64) results are summed on the host during
unsharding.
"""

import sys

sys.path.insert(0, "/opt/trn_rl_repo")

import numpy as np
import ml_dtypes

import concourse.bass as bass
import concourse.bacc as bacc
import concourse.mybir as mybir
from concourse import tile
from concourse.bass_utils import run_bass_kernel_spmd

N_CORES = 8
N_IN, N_REC, N_OUT = 512, 2048, 128
D_MAX = 4
B, T = 64, 250
M_LOC = N_REC // N_CORES          # 256 neurons per core
N_KT = N_REC // 128               # 16 k-tiles per delay tap
F32 = mybir.dt.float32
BF16 = mybir.dt.bfloat16
M2 = 2 * M_LOC                    # hi|lo packed rhs width (512)


def active_taps(t):
    # reference: tap d contributes only when t > d  (z_0 never feeds back)
    return [d for d in range(1, D_MAX + 1) if t > d]


def build_program(rec_c, out_c, thr, T_steps=T):
    nc = bacc.Bacc("TRN2", target_bir_lowering=False, debug=False,
                   num_devices=N_CORES)

    wd_in = nc.dram_tensor("wd", [4 * N_KT, 128, M2], BF16, kind="ExternalInput")
    wia_in = nc.dram_tensor("wia", [4, 128, M2], BF16, kind="ExternalInput")
    wib_in = nc.dram_tensor("wib", [4, 128, M_LOC], BF16, kind="ExternalInput")
    br_in = nc.dram_tensor("br", [1, M2], BF16, kind="ExternalInput")
    wo_in = nc.dram_tensor("wo", [128, 2 * 128], F32, kind="ExternalInput")
    id_in = nc.dram_tensor("ident", [64, 64], F32, kind="ExternalInput")
    psh_in = nc.dram_tensor("pshift", [128, 64], F32, kind="ExternalInput")
    xt_in = nc.dram_tensor("xt", [4, 128, T_steps, B], BF16, kind="ExternalInput")

    zs_out = nc.dram_tensor("zs", [T_steps, B, M_LOC], F32, kind="ExternalOutput")
    mo_out = nc.dram_tensor("mo", [128, B], F32, kind="ExternalOutput")

    with tile.TileContext(nc) as tc:
        with (
            tc.tile_pool(name="const", bufs=1) as cpool,
            tc.tile_pool(name="work", bufs=3) as wpool,
            tc.tile_pool(name="xtl", bufs=4) as xpool,
            tc.tile_pool(name="ps", bufs=2, space="PSUM") as pspool,
            tc.tile_pool(name="pst", bufs=2, space="PSUM") as ptpool,
            tc.tile_pool(name="psmo", bufs=1, space="PSUM") as pmopool,
            tc.tile_pool(name="dram", bufs=3, space="DRAM") as dpool,
        ):
            # ---- constants / persistent state ----
            wd_sb = cpool.tile([128, 4 * N_KT * M2], BF16, tag="wd_sb")
            for j in range(4 * N_KT):
                nc.sync.dma_start(wd_sb[:, j * M2:(j + 1) * M2], wd_in[j])
            wia_sb = cpool.tile([128, 4 * M2], BF16, tag="wia_sb")
            nc.sync.dma_start(wia_sb[:], wia_in.ap().rearrange("k p m -> p k m"))
            wib_sb = cpool.tile([128, 4 * M_LOC], BF16, tag="wib_sb")
            nc.sync.dma_start(wib_sb[:], wib_in.ap().rearrange("k p m -> p k m"))
            br_sb = cpool.tile([1, M2], BF16, tag="br_sb")
            nc.sync.dma_start(br_sb[:], br_in[:])
            wo_sb = cpool.tile([128, 2 * 128], F32, tag="wo_sb")
            nc.sync.dma_start(wo_sb[:], wo_in[:])
            id_sb = cpool.tile([64, 64], F32, tag="id_sb")
            nc.sync.dma_start(id_sb[:], id_in[:])
            ones_sb = cpool.tile([1, B], BF16, tag="ones_sb")
            nc.vector.memset(ones_sb[:], 1.0)

            mem = cpool.tile([B, M_LOC], F32, tag="mem")
            nc.vector.memset(mem[:], 0.0)
            wz = cpool.tile([B, M_LOC], F32, tag="wz")
            nc.vector.memset(wz[:], 0.0)
            # shift matrix for folding psum partitions 64-127 onto 0-63:
            # pshift[64+i, i] = 1  ->  out[m, n] = rhs[64+m, n]
            pshift = cpool.tile([128, 64], F32, tag="pshift")
            nc.sync.dma_start(pshift[:], psh_in[:])
            # fold staging buffers; rows 0-63 must stay zero forever
            folds = []
            for fi in range(2):
                fb = cpool.tile([128, M2], F32, tag=f"fold{fi}", name=f"fold{fi}")
                nc.vector.memset(fb[:], 0.0)
                folds.append(fb)
            # gathered spike ring: slot s holds z_t^T (neuron-major) for t%4==s
            zg = [cpool.tile([128, N_KT * B], BF16, tag=f"zg{s}", name=f"zg{s}")
                  for s in range(4)]

            for t in range(T_steps):
                taps = active_taps(t)
                # ---- PSUM accumulation: bias + input current + recurrent taps
                # The (128, 512) psum tile holds two column-group halves:
                # partitions 0-63 accumulate bias/input/even k-tiles, 64-127
                # the odd k-tiles (col-tiled concurrent matmuls).  Column
                # halves [0:256]/[256:512] are the bf16 hi/lo partial sums.
                ps = pspool.tile([128, M2], F32, tag="ps")
                nc.tensor.matmul(ps[0:64, :], ones_sb[0:1, :], br_sb[0:1, :],
                                 start=True, stop=False)
                xtile = xpool.tile([128, 4 * B], BF16, tag="xt")
                nc.scalar.dma_start(
                    xtile[:], xt_in[:, :, t, :].rearrange("k p b -> p k b"))
                for kt in range(4):
                    nc.tensor.matmul(ps[0:64, :], xtile[:, kt * B:(kt + 1) * B],
                                     wia_sb[:, kt * M2:(kt + 1) * M2],
                                     start=False, stop=False,
                                     tile_position=(0, 0))
                    nc.tensor.matmul(ps[0:64, 0:M_LOC],
                                     xtile[:, kt * B:(kt + 1) * B],
                                     wib_sb[:, kt * M_LOC:(kt + 1) * M_LOC],
                                     start=False, stop=(kt == 3 and not taps),
                                     tile_position=(0, 0))
                # recurrent taps: pair consecutive k-tiles on the two column
                # groups of the PE array so their streams run concurrently
                pairs = []
                for d in sorted(taps, reverse=True):    # d=1 (freshest) last
                    src = zg[(t - d) % 4]
                    for j in range(N_KT):
                        pairs.append((src, d, j))
                first_b = True
                for idx in range(0, len(pairs), 2):
                    for half, (src, d, j) in enumerate(pairs[idx:idx + 2]):
                        co = ((d - 1) * N_KT + j) * M2
                        last = idx + half == len(pairs) - 1
                        if half == 0:
                            nc.tensor.matmul(
                                ps[0:64, :], src[:, j * B:(j + 1) * B],
                                wd_sb[:, co:co + M2],
                                start=False, stop=last,
                                tile_position=(0, 0), skip_group_check=True)
                        else:
                            nc.tensor.matmul(
                                ps[64:128, :], src[:, j * B:(j + 1) * B],
                                wd_sb[:, co:co + M2],
                                start=first_b, stop=last,
                                tile_position=(0, 64), skip_group_check=True)
                            first_b = False

                # ---- fold odd-half psum partitions onto 0-63 via matmul ----
                have_b = not first_b
                if have_b:
                    fb = folds[t % 2]
                    nc.vector.tensor_copy(fb[64:128, :], ps[64:128, :])
                    nc.tensor.matmul(ps[0:64, :], pshift[:], fb[:],
                                     start=False, stop=True,
                                     tile_position=(0, 0), skip_group_check=True)

                # ---- membrane update / spike ----
                mem3 = wpool.tile([B, M_LOC], F32, tag="mem3")
                nc.vector.scalar_tensor_tensor(
                    mem3[:], mem[:], float(rec_c), ps[0:64, 0:M_LOC],
                    mybir.AluOpType.mult, mybir.AluOpType.add)
                nc.vector.tensor_add(mem3[:], mem3[:], ps[0:64, M_LOC:M2])
                z = wpool.tile([B, M_LOC], F32, tag="z")
                nc.vector.tensor_scalar(z[:], mem3[:], float(thr), None,
                                        mybir.AluOpType.is_gt)
                nc.vector.scalar_tensor_tensor(
                    mem[:], z[:], float(-thr), mem3[:],
                    mybir.AluOpType.mult, mybir.AluOpType.add)

                # ---- outputs fed from z ----
                nc.scalar.dma_start(zs_out[t], z[:])
                nc.vector.scalar_tensor_tensor(
                    wz[:], wz[:], float(out_c), z[:],
                    mybir.AluOpType.mult, mybir.AluOpType.add)

                # ---- spike exchange (needed for steps 1..T-2) ----
                if 1 <= t <= T_steps - 2:
                    pT = ptpool.tile([128, 128], F32, tag="pT")
                    nc.tensor.transpose(pT[:, 0:64], z[:, 0:128], id_sb[:])
                    nc.tensor.transpose(pT[:, 64:128], z[:, 128:256], id_sb[:])
                    zT = wpool.tile([128, 128], BF16, tag="zT")
                    nc.vector.tensor_copy(zT[:], pT[:])
                    bounce = dpool.tile([2 * 128, B], BF16, tag="bounce")
                    nc.sync.dma_start(
                        bounce[:].rearrange("(i p) b -> p i b", p=128), zT[:])
                    gb = dpool.tile([N_REC, B], BF16, tag="gb")
                    nc.gpsimd.collective_compute(
                        "AllGather", mybir.AluOpType.bypass,
                        replica_groups=[list(range(N_CORES))],
                        ins=[bounce.opt()], outs=[gb.opt()])
                    dst = zg[t % 4]
                    for g in range(4):
                        nc.gpsimd.dma_start(
                            dst[:, g * 4 * B:(g + 1) * 4 * B],
                            gb[g * 512:(g + 1) * 512, :]
                            .rearrange("(j p) b -> p j b", p=128))

            # ---- final output projection (fp32) ----
            pT2 = ptpool.tile([128, 128], F32, tag="pT")
            nc.tensor.transpose(pT2[:, 0:64], wz[:, 0:128], id_sb[:])
            nc.tensor.transpose(pT2[:, 64:128], wz[:, 128:256], id_sb[:])
            wzT = wpool.tile([128, 128], F32, tag="wzT")
            nc.vector.tensor_copy(wzT[:], pT2[:])
            pmo = pmopool.tile([128, B], F32, tag="pmo")
            nc.tensor.matmul(pmo[:], wo_sb[:, 0:128], wzT[:, 0:64],
                             start=True, stop=False)
            nc.tensor.matmul(pmo[:], wo_sb[:, 128:256], wzT[:, 64:128],
                             start=False, stop=True)
            mo_sb = wpool.tile([128, B], F32, tag="mo_sb")
            nc.vector.tensor_copy(mo_sb[:], pmo[:])
            nc.sync.dma_start(mo_out[:], mo_sb[:])

    nc.compile()
    return nc


def _split2(a):
    hi = a.astype(ml_dtypes.bfloat16)
    lo = (a - hi.astype(np.float32)).astype(ml_dtypes.bfloat16)
    return hi, lo


def prep_inputs(inputs, T_steps=T):
    """Host-side shard prep.  Returns (in_maps, scalars)."""
    x = np.ascontiguousarray(inputs["input_spike_raster"], dtype=np.float32)
    delays = np.asarray(inputs["delays"])
    W_i = np.asarray(inputs["W_i"], dtype=np.float32)
    W_r = np.asarray(inputs["W_r"], dtype=np.float32)
    W_o = np.asarray(inputs["W_o"], dtype=np.float32)
    B_r = np.asarray(inputs["B_r"], dtype=np.float32)
    tau_rec = float(np.asarray(inputs["tau_rec"]).reshape(-1)[0])
    tau_out = float(np.asarray(inputs["tau_out"]).reshape(-1)[0])
    thr = float(np.asarray(inputs["thr_rec"]).reshape(-1)[0])
    rec_c = float(np.exp(-0.001 * 10.0 / tau_rec))
    out_c = float(np.exp(-0.001 * 10.0 / tau_out))

    # xt[kt, p, t, b] = x[b, kt*128+p, t]   (binary -> bf16 exact; shared)
    xt = np.ascontiguousarray(
        x[:, :, :T_steps].transpose(1, 2, 0).reshape(4, 128, T_steps, B)
    ).astype(ml_dtypes.bfloat16)
    ident = np.eye(64, dtype=np.float32)

    in_maps = []
    for c in range(N_CORES):
        mlo = c * M_LOC
        msl = slice(mlo, mlo + M_LOC)
        # wd[(d-1)*16+j, p, 0:256|256:512] = hi|lo of
        #   W_r[mlo+m, j*128+p] * (delays[j*128+p, mlo+m]==d)
        Wsub = W_r[msl, :]                      # (256, 2048)
        Dsub = delays[:, msl]                   # (2048, 256)
        wd_f32 = np.zeros((4 * N_KT, 128, M_LOC), np.float32)
        for d in range(1, D_MAX + 1):
            Wmask = (Wsub.T * (Dsub == d)).astype(np.float32)   # (2048, 256)
            wd_f32[(d - 1) * N_KT:d * N_KT] = Wmask.reshape(N_KT, 128, M_LOC)
        wdh, wdl = _split2(wd_f32)
        wd = np.concatenate([wdh, wdl], axis=2)           # (64, 128, 512)
        # W_i 3-way split: wia = [hi|mid], wib = lo
        wi = np.ascontiguousarray(
            W_i[msl, :].T.reshape(4, 128, M_LOC))          # (4, 128, 256)
        wih = wi.astype(ml_dtypes.bfloat16)
        wim = wi - wih.astype(np.float32)
        wimh, wil = _split2(wim)
        wia = np.concatenate([wih, wimh], axis=2)          # (4, 128, 512)
        wib = wil                                          # (4, 128, 256)
        brh, brl = _split2(B_r[msl].reshape(1, M_LOC))
        br = np.concatenate([brh, brl], axis=1)            # (1, 512)
        # wo[p, i*128+o] = W_o[o, mlo + i*128 + p]
        wo = np.ascontiguousarray(
            W_o[:, msl].T.reshape(2, 128, N_OUT).transpose(1, 0, 2)
            .reshape(128, 2 * N_OUT))
        pshift = np.zeros((128, 64), np.float32)
        pshift[64 + np.arange(64), np.arange(64)] = 1.0
        in_maps.append({"wd": wd, "wia": wia, "wib": wib, "br": br,
                        "wo": wo, "ident": ident, "xt": xt,
                        "pshift": pshift})
    return in_maps, (rec_c, out_c, thr, tau_out)


def assemble_outputs(results, inputs, T_steps=T):
    B_o = np.asarray(inputs["B_o"], dtype=np.float32)
    tau_out = float(np.asarray(inputs["tau_out"]).reshape(-1)[0])
    out_c = np.exp(np.float32(-0.001 * 10.0 / tau_out), dtype=np.float32)

    zs = np.empty((B, N_REC, T_steps), np.float32)
    mo = np.zeros((128, B), np.float64)
    for c in range(N_CORES):
        zs[:, c * M_LOC:(c + 1) * M_LOC, :] = results[c]["zs"].transpose(1, 2, 0)
        mo += results[c]["mo"].astype(np.float64)
    # geometric B_o term: sum_{t=0..T-1} c^(T-1-t) * B_o
    geo = float((1.0 - np.float64(out_c) ** T_steps) / (1.0 - np.float64(out_c)))
    mem_out = (mo.T + geo * B_o[None, :].astype(np.float64)).astype(np.float32)
    return mem_out, zs


_CACHED = {}


def kernel(**inputs):
    T_steps = T
    in_maps, (rec_c, out_c, thr, _) = prep_inputs(inputs, T_steps)
    key = (round(rec_c, 10), round(out_c, 10), round(thr, 10), T_steps)
    if key not in _CACHED:
        _CACHED[key] = build_program(rec_c, out_c, thr, T_steps)
    nc = _CACHED[key]
    res = run_bass_kernel_spmd(nc, in_maps, list(range(N_CORES)))
    return assemble_outputs(res.results, inputs, T_steps)


if __name__ == "__main__":
    rng = np.random.RandomState(0)
    ins = {
        "input_spike_raster": (rng.rand(B, N_IN, T) < 0.05).astype(np.float32),
        "delays": rng.randint(0, 5, (N_REC, N_REC)).astype(np.int32),
        "W_i": rng.randn(N_REC, N_IN).astype(np.float32) * 0.01,
        "W_r": rng.randn(N_REC, N_REC).astype(np.float32) * 0.002,
        "W_o": rng.randn(N_OUT, N_REC).astype(np.float32) * 0.01,
        "B_r": np.zeros(N_REC, np.float32),
        "B_o": np.zeros(N_OUT, np.float32),
        "tau_rec": np.full(1, 0.2, np.float32),
        "tau_out": np.full(1, 0.2, np.float32),
        "thr_rec": np.full(1, 0.5, np.float32),
    }
    mo, zs = kernel(**ins)
    print("mo", mo.shape, "zs", zs.shape, zs.mean())


# revision 21
# speedup vs baseline: 1.0371x; 1.0371x over previous
"""Trainium2 Bass kernel for the DelayNetwork spiking RNN.

Strategy: tensor-parallel over the 2048 recurrent neurons across 8 cores
(256 neurons each).  Each core keeps its slice of the 4 delay-masked
recurrent weight matrices resident in SBUF, split into bf16 hi+lo pairs
(W = W_hi + W_lo); spikes are exactly representable in bf16, so two
full-rate bf16 matmuls reproduce the f32 matmul to ~1e-7 absolute — at
2x the fp32 PE rate (fp32 matmul runs at 4 cycles/row on TRN2, bf16 at
1).  The hi and lo halves are packed side-by-side in one N=512 rhs so
each k-tile costs a single LDWEIGHTS+MATMUL pair; the PSUM column halves
are summed in the membrane update.  W_i is split three ways (its values
are ~20x larger, so a 2-way split leaves ~2e-6 current error that flips
spikes).

Per step: spike-stationary matmuls (lhsT = gathered spike k-tile
(128,64) bf16) accumulate bias + input current + 4 delay taps into one
f32 PSUM group; a fused DVE chain updates the membrane and thresholds
spikes; the spike vector is PE-transposed to neuron-major, cast to bf16,
and AllGathered (d>=2 taps of the next step overlap the collective);
the spike raster streams to DRAM.  The output projection is deferred:
mem_out(T) = sum_t c^(T-1-t) (z_t @ W_o^T + B_o), accumulated as
wz = wz*c + z_t per step with a single 2-matmul fp32 projection at the
end; the 8 partial (128,64) results are summed on the host during
unsharding.
"""

import sys

sys.path.insert(0, "/opt/trn_rl_repo")

import numpy as np
import ml_dtypes

import concourse.bass as bass
import concourse.bacc as bacc
import concourse.mybir as mybir
from concourse import tile
from concourse.bass_utils import run_bass_kernel_spmd

N_CORES = 8
N_IN, N_REC, N_OUT = 512, 2048, 128
D_MAX = 4
B, T = 64, 250
M_LOC = N_REC // N_CORES          # 256 neurons per core
N_KT = N_REC // 128               # 16 k-tiles per delay tap
F32 = mybir.dt.float32
BF16 = mybir.dt.bfloat16
M2 = 2 * M_LOC                    # hi|lo packed rhs width (512)


def active_taps(t):
    # reference: tap d contributes only when t > d  (z_0 never feeds back)
    return [d for d in range(1, D_MAX + 1) if t > d]


def build_program(rec_c, out_c, thr, T_steps=T):
    nc = bacc.Bacc("TRN2", target_bir_lowering=False, debug=False,
                   num_devices=N_CORES)

    wd_in = nc.dram_tensor("wd", [4 * N_KT, 128, M2], BF16, kind="ExternalInput")
    wia_in = nc.dram_tensor("wia", [4, 128, M2], BF16, kind="ExternalInput")
    wib_in = nc.dram_tensor("wib", [4, 128, M_LOC], BF16, kind="ExternalInput")
    br_in = nc.dram_tensor("br", [1, M2], BF16, kind="ExternalInput")
    wo_in = nc.dram_tensor("wo", [128, 2 * 128], F32, kind="ExternalInput")
    id_in = nc.dram_tensor("ident", [64, 64], F32, kind="ExternalInput")
    psh_in = nc.dram_tensor("pshift", [128, 64], F32, kind="ExternalInput")
    xt_in = nc.dram_tensor("xt", [4, 128, T_steps, B], BF16, kind="ExternalInput")

    zs_out = nc.dram_tensor("zs", [T_steps, B, M_LOC], F32, kind="ExternalOutput")
    mo_out = nc.dram_tensor("mo", [128, B], F32, kind="ExternalOutput")

    with tile.TileContext(nc) as tc:
        with (
            tc.tile_pool(name="const", bufs=1) as cpool,
            tc.tile_pool(name="work", bufs=3) as wpool,
            tc.tile_pool(name="xtl", bufs=4) as xpool,
            tc.tile_pool(name="ps", bufs=2, space="PSUM") as pspool,
            tc.tile_pool(name="pst", bufs=2, space="PSUM") as ptpool,
            tc.tile_pool(name="psmo", bufs=1, space="PSUM") as pmopool,
            tc.tile_pool(name="dram", bufs=3, space="DRAM") as dpool,
        ):
            # ---- constants / persistent state ----
            wd_sb = cpool.tile([128, 4 * N_KT * M2], BF16, tag="wd_sb")
            for j in range(4 * N_KT):
                nc.sync.dma_start(wd_sb[:, j * M2:(j + 1) * M2], wd_in[j])
            wia_sb = cpool.tile([128, 4 * M2], BF16, tag="wia_sb")
            nc.sync.dma_start(wia_sb[:], wia_in.ap().rearrange("k p m -> p k m"))
            wib_sb = cpool.tile([128, 4 * M_LOC], BF16, tag="wib_sb")
            nc.sync.dma_start(wib_sb[:], wib_in.ap().rearrange("k p m -> p k m"))
            br_sb = cpool.tile([1, M2], BF16, tag="br_sb")
            nc.sync.dma_start(br_sb[:], br_in[:])
            wo_sb = cpool.tile([128, 2 * 128], F32, tag="wo_sb")
            nc.sync.dma_start(wo_sb[:], wo_in[:])
            id_sb = cpool.tile([64, 64], F32, tag="id_sb")
            nc.sync.dma_start(id_sb[:], id_in[:])
            ones_sb = cpool.tile([1, B], BF16, tag="ones_sb")
            nc.vector.memset(ones_sb[:], 1.0)

            mem = cpool.tile([B, M_LOC], F32, tag="mem")
            nc.vector.memset(mem[:], 0.0)
            wz = cpool.tile([B, M_LOC], F32, tag="wz")
            nc.vector.memset(wz[:], 0.0)
            # shift matrix for folding psum partitions 64-127 onto 0-63:
            # pshift[64+i, i] = 1  ->  out[m, n] = rhs[64+m, n]
            pshift = cpool.tile([128, 64], F32, tag="pshift")
            nc.sync.dma_start(pshift[:], psh_in[:])
            # fold staging buffers; rows 0-63 must stay zero forever
            folds = []
            for fi in range(2):
                fb = cpool.tile([128, M2], F32, tag=f"fold{fi}", name=f"fold{fi}")
                nc.vector.memset(fb[:], 0.0)
                folds.append(fb)
            # gathered spike ring: slot s holds z_t^T (neuron-major) for t%4==s
            zg = [cpool.tile([128, N_KT * B], BF16, tag=f"zg{s}", name=f"zg{s}")
                  for s in range(4)]

            def emit_pairs(ps, pairs, first_b, final):
                # col-tiled pairs: even index -> array cols 0-63 / psum
                # partitions 0-63, odd -> cols 64-127 / partitions 64-127,
                # so the two weight streams run concurrently.
                for idx in range(0, len(pairs), 2):
                    for half, (src, d, j) in enumerate(pairs[idx:idx + 2]):
                        co = ((d - 1) * N_KT + j) * M2
                        last = final and (idx + half == len(pairs) - 1)
                        if half == 0:
                            nc.tensor.matmul(
                                ps[0:64, :], src[:, j * B:(j + 1) * B],
                                wd_sb[:, co:co + M2],
                                start=False, stop=last,
                                tile_position=(0, 0), skip_group_check=True)
                        else:
                            nc.tensor.matmul(
                                ps[64:128, :], src[:, j * B:(j + 1) * B],
                                wd_sb[:, co:co + M2],
                                start=first_b, stop=last,
                                tile_position=(0, 64), skip_group_check=True)
                            first_b = False
                return first_b

            def emit_incur(t):
                """Bias + input-current matmuls for step t (no AG dep)."""
                taps = active_taps(t)
                ps = pspool.tile([128, M2], F32, tag="ps", name=f"ps{t % 2}")
                nc.tensor.matmul(ps[0:64, :], ones_sb[0:1, :], br_sb[0:1, :],
                                 start=True, stop=False)
                xtile = xpool.tile([128, 4 * B], BF16, tag="xt", name=f"xtl{t % 4}")
                nc.scalar.dma_start(
                    xtile[:], xt_in[:, :, t, :].rearrange("k p b -> p k b"))
                no_more = not taps
                for kt in range(4):
                    nc.tensor.matmul(ps[0:64, :], xtile[:, kt * B:(kt + 1) * B],
                                     wia_sb[:, kt * M2:(kt + 1) * M2],
                                     start=False, stop=False,
                                     tile_position=(0, 0))
                    nc.tensor.matmul(ps[0:64, 0:M_LOC],
                                     xtile[:, kt * B:(kt + 1) * B],
                                     wib_sb[:, kt * M_LOC:(kt + 1) * M_LOC],
                                     start=False, stop=(kt == 3 and no_more),
                                     tile_position=(0, 0))
                return ps

            def emit_d432(t, ps):
                """The d>=2 tap matmuls for step t (no AG dep either —
                emitted after step t-1's exchange so the PE has work queued
                under the in-flight collective)."""
                taps = active_taps(t)
                pairs = [(zg[(t - d) % 4], d, j)
                         for d in sorted(taps, reverse=True) if d >= 2
                         for j in range(N_KT)]
                return emit_pairs(ps, pairs, True, final=False)

            ps_cur = emit_incur(0)
            fb_flag = emit_d432(0, ps_cur)
            for t in range(T_steps):
                taps = active_taps(t)
                ps = ps_cur
                # ---- phase 2: the d=1 tap (waits on the z_{t-1} gather) ----
                if 1 in taps:
                    d1_pairs = [(zg[(t - 1) % 4], 1, j) for j in range(N_KT)]
                    fb_flag = emit_pairs(ps, d1_pairs, fb_flag, final=True)

                # ---- fold odd-half psum partitions onto 0-63 via matmul ----
                if not fb_flag:      # some B-half matmuls were emitted
                    fb = folds[t % 2]
                    nc.vector.tensor_copy(fb[64:128, :], ps[64:128, :])
                    nc.tensor.matmul(ps[0:64, :], pshift[:], fb[:],
                                     start=False, stop=True,
                                     tile_position=(0, 0), skip_group_check=True)

                # ---- membrane update / spike ----
                mem3 = wpool.tile([B, M_LOC], F32, tag="mem3")
                nc.vector.scalar_tensor_tensor(
                    mem3[:], mem[:], float(rec_c), ps[0:64, 0:M_LOC],
                    mybir.AluOpType.mult, mybir.AluOpType.add)
                nc.vector.tensor_add(mem3[:], mem3[:], ps[0:64, M_LOC:M2])
                z = wpool.tile([B, M_LOC], F32, tag="z")
                nc.vector.tensor_scalar(z[:], mem3[:], float(thr), None,
                                        mybir.AluOpType.is_gt)
                nc.vector.scalar_tensor_tensor(
                    mem[:], z[:], float(-thr), mem3[:],
                    mybir.AluOpType.mult, mybir.AluOpType.add)

                # ---- outputs fed from z ----
                nc.scalar.dma_start(zs_out[t], z[:])
                nc.vector.scalar_tensor_tensor(
                    wz[:], wz[:], float(out_c), z[:],
                    mybir.AluOpType.mult, mybir.AluOpType.add)

                # prefetch next step's input-current matmuls so the PE has
                # work to chew on while this step's AllGather is in flight
                ps_next = emit_incur(t + 1) if t + 1 < T_steps else None

                # ---- spike exchange (needed for steps 1..T-2) ----
                if 1 <= t <= T_steps - 2:
                    pT = ptpool.tile([128, 128], F32, tag="pT")
                    nc.tensor.transpose(pT[:, 0:64], z[:, 0:128], id_sb[:])
                    nc.tensor.transpose(pT[:, 64:128], z[:, 128:256], id_sb[:])
                    zT = wpool.tile([128, 128], BF16, tag="zT")
                    nc.vector.tensor_copy(zT[:], pT[:])
                    bounce = dpool.tile([2 * 128, B], BF16, tag="bounce")
                    nc.sync.dma_start(
                        bounce[:].rearrange("(i p) b -> p i b", p=128), zT[:])
                    gb = dpool.tile([N_REC, B], BF16, tag="gb")
                    nc.gpsimd.collective_compute(
                        "AllGather", mybir.AluOpType.bypass,
                        replica_groups=[list(range(N_CORES))],
                        ins=[bounce.opt()], outs=[gb.opt()])
                    dst = zg[t % 4]
                    for g in range(4):
                        nc.sync.dma_start(
                            dst[:, g * 4 * B:(g + 1) * 4 * B],
                            gb[g * 512:(g + 1) * 512, :]
                            .rearrange("(j p) b -> p j b", p=128))

                if ps_next is not None:
                    fb_flag = emit_d432(t + 1, ps_next)
                    ps_cur = ps_next

            # ---- final output projection (fp32) ----
            pT2 = ptpool.tile([128, 128], F32, tag="pT")
            nc.tensor.transpose(pT2[:, 0:64], wz[:, 0:128], id_sb[:])
            nc.tensor.transpose(pT2[:, 64:128], wz[:, 128:256], id_sb[:])
            wzT = wpool.tile([128, 128], F32, tag="wzT")
            nc.vector.tensor_copy(wzT[:], pT2[:])
            pmo = pmopool.tile([128, B], F32, tag="pmo")
            nc.tensor.matmul(pmo[:], wo_sb[:, 0:128], wzT[:, 0:64],
                             start=True, stop=False)
            nc.tensor.matmul(pmo[:], wo_sb[:, 128:256], wzT[:, 64:128],
                             start=False, stop=True)
            mo_sb = wpool.tile([128, B], F32, tag="mo_sb")
            nc.vector.tensor_copy(mo_sb[:], pmo[:])
            nc.sync.dma_start(mo_out[:], mo_sb[:])

    nc.compile()
    return nc


def _split2(a):
    hi = a.astype(ml_dtypes.bfloat16)
    lo = (a - hi.astype(np.float32)).astype(ml_dtypes.bfloat16)
    return hi, lo


def prep_inputs(inputs, T_steps=T):
    """Host-side shard prep.  Returns (in_maps, scalars)."""
    x = np.ascontiguousarray(inputs["input_spike_raster"], dtype=np.float32)
    delays = np.asarray(inputs["delays"])
    W_i = np.asarray(inputs["W_i"], dtype=np.float32)
    W_r = np.asarray(inputs["W_r"], dtype=np.float32)
    W_o = np.asarray(inputs["W_o"], dtype=np.float32)
    B_r = np.asarray(inputs["B_r"], dtype=np.float32)
    tau_rec = float(np.asarray(inputs["tau_rec"]).reshape(-1)[0])
    tau_out = float(np.asarray(inputs["tau_out"]).reshape(-1)[0])
    thr = float(np.asarray(inputs["thr_rec"]).reshape(-1)[0])
    rec_c = float(np.exp(-0.001 * 10.0 / tau_rec))
    out_c = float(np.exp(-0.001 * 10.0 / tau_out))

    # xt[kt, p, t, b] = x[b, kt*128+p, t]   (binary -> bf16 exact; shared)
    xt = np.ascontiguousarray(
        x[:, :, :T_steps].transpose(1, 2, 0).reshape(4, 128, T_steps, B)
    ).astype(ml_dtypes.bfloat16)
    ident = np.eye(64, dtype=np.float32)

    in_maps = []
    for c in range(N_CORES):
        mlo = c * M_LOC
        msl = slice(mlo, mlo + M_LOC)
        # wd[(d-1)*16+j, p, 0:256|256:512] = hi|lo of
        #   W_r[mlo+m, j*128+p] * (delays[j*128+p, mlo+m]==d)
        Wsub = W_r[msl, :]                      # (256, 2048)
        Dsub = delays[:, msl]                   # (2048, 256)
        wd_f32 = np.zeros((4 * N_KT, 128, M_LOC), np.float32)
        for d in range(1, D_MAX + 1):
            Wmask = (Wsub.T * (Dsub == d)).astype(np.float32)   # (2048, 256)
            wd_f32[(d - 1) * N_KT:d * N_KT] = Wmask.reshape(N_KT, 128, M_LOC)
        wdh, wdl = _split2(wd_f32)
        wd = np.concatenate([wdh, wdl], axis=2)           # (64, 128, 512)
        # W_i 3-way split: wia = [hi|mid], wib = lo
        wi = np.ascontiguousarray(
            W_i[msl, :].T.reshape(4, 128, M_LOC))          # (4, 128, 256)
        wih = wi.astype(ml_dtypes.bfloat16)
        wim = wi - wih.astype(np.float32)
        wimh, wil = _split2(wim)
        wia = np.concatenate([wih, wimh], axis=2)          # (4, 128, 512)
        wib = wil                                          # (4, 128, 256)
        brh, brl = _split2(B_r[msl].reshape(1, M_LOC))
        br = np.concatenate([brh, brl], axis=1)            # (1, 512)
        # wo[p, i*128+o] = W_o[o, mlo + i*128 + p]
        wo = np.ascontiguousarray(
            W_o[:, msl].T.reshape(2, 128, N_OUT).transpose(1, 0, 2)
            .reshape(128, 2 * N_OUT))
        pshift = np.zeros((128, 64), np.float32)
        pshift[64 + np.arange(64), np.arange(64)] = 1.0
        in_maps.append({"wd": wd, "wia": wia, "wib": wib, "br": br,
                        "wo": wo, "ident": ident, "xt": xt,
                        "pshift": pshift})
    return in_maps, (rec_c, out_c, thr, tau_out)


def assemble_outputs(results, inputs, T_steps=T):
    B_o = np.asarray(inputs["B_o"], dtype=np.float32)
    tau_out = float(np.asarray(inputs["tau_out"]).reshape(-1)[0])
    out_c = np.exp(np.float32(-0.001 * 10.0 / tau_out), dtype=np.float32)

    zs = np.empty((B, N_REC, T_steps), np.float32)
    mo = np.zeros((128, B), np.float64)
    for c in range(N_CORES):
        zs[:, c * M_LOC:(c + 1) * M_LOC, :] = results[c]["zs"].transpose(1, 2, 0)
        mo += results[c]["mo"].astype(np.float64)
    # geometric B_o term: sum_{t=0..T-1} c^(T-1-t) * B_o
    geo = float((1.0 - np.float64(out_c) ** T_steps) / (1.0 - np.float64(out_c)))
    mem_out = (mo.T + geo * B_o[None, :].astype(np.float64)).astype(np.float32)
    return mem_out, zs


_CACHED = {}


def kernel(**inputs):
    T_steps = T
    in_maps, (rec_c, out_c, thr, _) = prep_inputs(inputs, T_steps)
    key = (round(rec_c, 10), round(out_c, 10), round(thr, 10), T_steps)
    if key not in _CACHED:
        _CACHED[key] = build_program(rec_c, out_c, thr, T_steps)
    nc = _CACHED[key]
    res = run_bass_kernel_spmd(nc, in_maps, list(range(N_CORES)))
    return assemble_outputs(res.results, inputs, T_steps)


if __name__ == "__main__":
    rng = np.random.RandomState(0)
    ins = {
        "input_spike_raster": (rng.rand(B, N_IN, T) < 0.05).astype(np.float32),
        "delays": rng.randint(0, 5, (N_REC, N_REC)).astype(np.int32),
        "W_i": rng.randn(N_REC, N_IN).astype(np.float32) * 0.01,
        "W_r": rng.randn(N_REC, N_REC).astype(np.float32) * 0.002,
        "W_o": rng.randn(N_OUT, N_REC).astype(np.float32) * 0.01,
        "B_r": np.zeros(N_REC, np.float32),
        "B_o": np.zeros(N_OUT, np.float32),
        "tau_rec": np.full(1, 0.2, np.float32),
        "tau_out": np.full(1, 0.2, np.float32),
        "thr_rec": np.full(1, 0.5, np.float32),
    }
    mo, zs = kernel(**ins)
    print("mo", mo.shape, "zs", zs.shape, zs.mean())
